# revision 8
# baseline (speedup 1.0000x reference)
"""Deformable conv (DCNv2) Trainium2 Bass kernel — ap_gather edition.

Problem (hardcoded): x [8, 128, 64, 64] f32; offset/mask 3x3 convs (pad 1);
bilinear-gather im2col; GEMM with weights [256, 1152]; out [8, 256, 64, 64].

Sharding: data-parallel over batch N=8 across 8 NeuronCores (1 sample/core);
weights/conv params replicated.

Per-core pipeline (sample n):
  1. x -> SBUF, cast bf16, build zero-padded xpad [128, 68*68] (ring 2) and a
     quad-expanded image xq[c, px, 0:4] = [x(y,x), x(y,x+1), x(y+1,x),
     x(y+1,x+1)] (px = y*68+x) — all in SBUF, nothing staged in DRAM.
  2. PE: offset/mask conv as 9 shifted matmuls (27 rows: dy 0-8, dx 9-17,
     mask 18-26); PE-transpose to j-major [jp, q, t] (j = t*128 + jp).
  3. DVE coordinate math in j-major: floor via round-trick, clamp to
     [-2, 64], one quad index per (k, j), mask*bilinear corner weights
     wq[jp, k, t, r] (r = corner).
  4. Index wrap for ap_gather uses gather order i = jp*32 + t: PE-transpose
     idx chunks, i16 wrapped16 [16, k, m] -> DRAM -> broadcast x8 replicas.
     Weights flatten per tap to one partition (wflat[k, i*4+r], 128 descs of
     256B) for partition-broadcast DMA.
  5. Main loop over 4 i-chunks x 9 taps: gpsimd.ap_gather pulls quads
     [128c, 1024, 4] straight from SBUF in channel-major order; corner
     weights partition-broadcast via SBUF->SBUF DMA; DVE weighted-reduce to
     col_k; PE GEMM accumulates (start/stop over k) into PSUM; bias add;
     f32 out in i-order (host unpermutes jp/t).
"""

import numpy as np
import ml_dtypes

import concourse.bass as bass
import concourse.mybir as mybir
import concourse.tile as tile
from concourse import bacc
from concourse.bass_utils import run_bass_kernel_spmd
from concourse.masks import make_identity

F32 = mybir.dt.float32
BF16 = mybir.dt.bfloat16
I16 = mybir.dt.int16

N, C, H, W = 8, 128, 64, 64
K, K2, P = 3, 9, 256
HW = H * W                  # 4096
PW = W + 4                  # 68  (pad ring of 2)
ROWS = PW * PW              # 4624
NT = HW // 128              # 32 t-values
KT = K2 * NT                # 288
NCH = 4                     # i-chunks
CH = HW // NCH              # 1024 idx per gather
MAGIC = 12582912.0          # 1.5 * 2**23: fp32 round-to-int magic

_CACHE = {}


def _build_nc():
    nc = bacc.Bacc("TRN2", target_bir_lowering=False, debug=False,
                   num_devices=N, num_swdge_queues=4)

    x_in = nc.dram_tensor("x", [C, HW], F32, kind="ExternalInput")
    lhsT_om = nc.dram_tensor("lhsT_om", [C, K2, 32], BF16, kind="ExternalInput")
    lhsT_gemm = nc.dram_tensor("lhsT_gemm", [C, K2, P], BF16, kind="ExternalInput")
    basey = nc.dram_tensor("basey", [128, KT], F32, kind="ExternalInput")
    basex = nc.dram_tensor("basex", [128, KT], F32, kind="ExternalInput")
    bias_col = nc.dram_tensor("bias_col", [128, 2], F32, kind="ExternalInput")
    y_out = nc.dram_tensor("y", [P, HW], F32, kind="ExternalOutput")

    with tile.TileContext(nc) as tc:
        with tc.tile_pool(name="dram", bufs=1, space="DRAM") as dram:
            idxw_dram = dram.tile([16, K2 * 256], I16)
            wflat_dram = dram.tile([K2, 4 * HW], BF16)
            _emit(tc, nc, x_in, lhsT_om, lhsT_gemm, basey, basex,
                  bias_col, y_out, idxw_dram, wflat_dram)
    nc.compile()
    return nc


def _emit(tc, nc, x_in, lhsT_om, lhsT_gemm, basey, basex, bias_col,
          y_out, idxw_dram, wflat_dram):
    TS = nc.vector.tensor_scalar
    TT_ADD = nc.vector.tensor_add
    TT_SUB = nc.vector.tensor_sub
    TT_MUL = nc.vector.tensor_mul
    Alu = mybir.AluOpType

    with tc.tile_pool(name="singles", bufs=1) as singles:
        # ---- persistent tiles ----
        om_sb = singles.tile([C, K2, 32], BF16, tag="om", name="om")
        gemm_sb = singles.tile([C, K2, P], BF16, tag="gemm_w", name="gemm_w")
        bias_sb = singles.tile([128, 2], F32, tag="bias", name="bias")
        identf32 = singles.tile([128, 128], F32, tag="idf32", name="idf32")
        identf = singles.tile([32, 32], F32, tag="identf", name="identf")
        xq = singles.tile([C, ROWS, 4], BF16, tag="xq", name="xq")
        xpad = singles.tile([C, ROWS + 72], BF16, tag="xpad", name="xpad")
        idx_sb = singles.tile([128, K2, 256], I16, tag="idx_sb", name="idx_sb")

        nc.sync.dma_start(out=om_sb, in_=lhsT_om[:])
        nc.sync.dma_start(out=gemm_sb, in_=lhsT_gemm[:])
        nc.sync.dma_start(out=bias_sb, in_=bias_col[:])
        make_identity(nc, identf32)
        make_identity(nc, identf)

        with tc.tile_pool(name="stage1", bufs=1) as st1, \
             tc.tile_pool(name="coord", bufs=1) as coord, \
             tc.tile_pool(name="ps_a", bufs=2, space="PSUM") as ps_a:

            # ---- stage 1: load x, cast, build xpad + xq (all SBUF) ----
            x_sb = st1.tile([C, HW], F32, tag="x", name="x")
            nc.sync.dma_start(out=x_sb, in_=x_in[:])
            nc.vector.memset(xpad, 0.0)
            xpad_int = bass.AP(tensor=xpad.tensor,
                               offset=xpad.offset + 2 * PW + 2,
                               ap=[xpad.ap[0], [PW, H], [1, W]])
            nc.scalar.copy(out=xpad_int,
                           in_=x_sb[:].rearrange("c (h w) -> c h w", h=H))
            # quad expansion: xq[:, px, q] = xpad[:, px + off_q]
            for q, off in enumerate((0, 1, PW, PW + 1)):
                dst = bass.AP(tensor=xq.tensor, offset=xq.offset + q,
                              ap=[xq.ap[0], [4, ROWS]])
                src = bass.AP(tensor=xpad.tensor, offset=xpad.offset + off,
                              ap=[xpad.ap[0], [1, ROWS]])
                if q < 2:
                    nc.scalar.copy(out=dst, in_=src)
                else:
                    nc.vector.tensor_copy(dst, src)

            # ---- stage 2: offset/mask conv (27 out rows), 512-col chunks ----
            co_sb = st1.tile([32, HW], F32, tag="co", name="co")
            for nt8 in range(8):
                co_ps = ps_a.tile([32, 512], F32, tag="conv", name="conv")
                for tap in range(K2):
                    dy, dx = tap // K, tap % K
                    rhs = bass.AP(
                        tensor=xpad.tensor,
                        offset=(xpad.offset + (1 + dy) * PW + (1 + dx)
                                + (nt8 * 8) * PW),
                        ap=[xpad.ap[0], [PW, 8], [1, W]],
                    )
                    nc.tensor.matmul(co_ps[:], om_sb[:, tap, :], rhs,
                                     start=(tap == 0), stop=(tap == K2 - 1))
                nc.scalar.copy(out=co_sb[:, nt8 * 512:(nt8 + 1) * 512],
                               in_=co_ps)

            # ---- stage 3: transpose conv out to j-major [jp, q, t] ----
            trj = coord.tile([128, 32, NT], F32, tag="trj", name="trj")
            for t in range(NT):
                tp = ps_a.tile([128, 32], F32, tag="trjp", name="trjp")
                nc.tensor.transpose(tp[:], co_sb[:, t * 128:(t + 1) * 128],
                                    identf[:])
                nc.vector.tensor_copy(trj[:, :, t], tp)

            dy_all = trj[:, 0:K2, :]
            dx_all = trj[:, K2:2 * K2, :]
            m_all = trj[:, 2 * K2:3 * K2, :]

            by = coord.tile([128, KT], F32, tag="by", name="by")
            bx = coord.tile([128, KT], F32, tag="bx", name="bx")
            nc.sync.dma_start(out=by, in_=basey[:])
            nc.sync.dma_start(out=bx, in_=basex[:])

            def f32t(tag):
                return coord.tile([128, KT], F32, tag=tag, name=tag)

            py = f32t("py"); TT_ADD(py, dy_all, by)
            px = f32t("px"); TT_ADD(px, dx_all, bx)
            ty = f32t("ty"); TS(out=ty, in0=py, scalar1=-0.5, scalar2=MAGIC,
                                op0=Alu.add, op1=Alu.add)
            y0 = f32t("y0"); TS(out=y0, in0=ty, scalar1=MAGIC, scalar2=None,
                                op0=Alu.subtract)
            tx = f32t("tx"); TS(out=tx, in0=px, scalar1=-0.5, scalar2=MAGIC,
                                op0=Alu.add, op1=Alu.add)
            x0 = f32t("x0"); TS(out=x0, in0=tx, scalar1=MAGIC, scalar2=None,
                                op0=Alu.subtract)
            ly = f32t("ly"); TT_SUB(ly, py, y0)
            lx = f32t("lx"); TT_SUB(lx, px, x0)
            y0c = f32t("y0c"); TS(out=y0c, in0=y0, scalar1=-2.0, scalar2=64.0,
                                  op0=Alu.max, op1=Alu.min)
            x0c = f32t("x0c"); TS(out=x0c, in0=x0, scalar1=-2.0, scalar2=64.0,
                                  op0=Alu.max, op1=Alu.min)

            # quad index (one per (k, j)): (y0c+2)*68 + (x0c+2)
            ia = f32t("ia"); TS(out=ia, in0=y0c, scalar1=float(PW),
                                scalar2=float(2 * PW + 2),
                                op0=Alu.mult, op1=Alu.add)
            idxf = f32t("idxf"); TT_ADD(idxf, ia, x0c)

            # mask * bilinear corner weights (mask = 2*sigmoid(conv))
            sig = f32t("sig")
            nc.scalar.activation(out=sig, in_=m_all,
                                 func=mybir.ActivationFunctionType.Sigmoid)
            m2 = f32t("m2"); TS(out=m2, in0=sig, scalar1=2.0, scalar2=None,
                                op0=Alu.mult)
            mly = f32t("mly"); TT_MUL(mly, m2, ly)
            muy = f32t("muy"); TT_SUB(muy, m2, mly)
            w11 = f32t("w11"); TT_MUL(w11, mly, lx)
            w10 = f32t("w10"); TT_SUB(w10, mly, w11)
            w01 = f32t("w01"); TT_MUL(w01, muy, lx)
            w00 = f32t("w00"); TT_SUB(w00, muy, w01)

            # wq[jp, k, t, r] bf16 (r innermost)
            wq = coord.tile([128, K2, NT, 4], BF16, tag="wq", name="wq")
            for r, wt in enumerate((w00, w01, w10, w11)):
                dst = bass.AP(tensor=wq.tensor, offset=wq.offset + r,
                              ap=[wq.ap[0], [4 * NT, K2], [4, NT]])
                nc.vector.tensor_copy(dst, wt[:].rearrange("p (k t) -> p k t",
                                                           k=K2))

            # wflat[k, i*4 + r] = wq[jp, k, t, r], i = jp*32 + t
            # (one 256B desc per (k, jp): src (t, r) run -> dst offset jp*128)
            for k in range(K2):
                src = bass.AP(tensor=wq.tensor, offset=wq.offset + k * 4 * NT,
                              ap=[wq.ap[0], [1, 4 * NT]])
                dst = bass.AP(tensor=wflat_dram.tensor,
                              offset=wflat_dram.offset + k * 4 * HW,
                              ap=[[4 * NT, 128], [1, 4 * NT]])
                nc.sync.dma_start(out=dst, in_=src)

            # idx wrap: wrapped16[t%16, k, 2*jp + t//16] = idxf[jp, k, t]
            wrapped16 = coord.tile([16, K2, 256], I16, tag="wr16", name="wr16")
            for k in range(K2):
                for b in range(2):
                    tpi = ps_a.tile([16, 128], F32, tag="tpi", name="tpi")
                    nc.tensor.transpose(
                        tpi[:], idxf[:, k * NT + 16 * b:k * NT + 16 * (b + 1)],
                        identf32[:])
                    dst = bass.AP(tensor=wrapped16.tensor,
                                  offset=wrapped16.offset + k * 256 + b,
                                  ap=[wrapped16.ap[0], [2, 128]])
                    nc.vector.tensor_copy(dst, tpi)
            nc.sync.dma_start(out=idxw_dram[:], in_=wrapped16)
            bsrc = bass.AP(tensor=idxw_dram.tensor, offset=idxw_dram.offset,
                           ap=[[0, 8], [K2 * 256, 16], [1, K2 * 256]])
            idst = bass.AP(tensor=idx_sb.tensor, offset=idx_sb.offset,
                           ap=[[idx_sb.ap[0][0], 128], [1, K2 * 256]])
            nc.sync.dma_start(out=idst, in_=bsrc)

        # ---- main loop: per i-chunk, per tap: gather/weight/GEMM ----
        with tc.tile_pool(name="gw", bufs=2) as gw, \
             tc.tile_pool(name="colp", bufs=2) as colp, \
             tc.tile_pool(name="outp", bufs=2) as outp, \
             tc.tile_pool(name="ps_b", bufs=2, space="PSUM") as ps_b:

            for c0 in range(NCH):
                ps = [[ps_b.tile([128, 512], F32, tag=f"g{m}{h}",
                                 name=f"g{m}{h}") for h in range(2)]
                      for m in range(2)]
                for k in range(K2):
                    g = gw.tile([128, CH, 4], BF16, tag="g", name="g")
                    nc.gpsimd.ap_gather(
                        out_ap=g[:],
                        in_ap=xq[:],
                        idxs_ap=idx_sb[:, k, c0 * (CH // 16):
                                       (c0 + 1) * (CH // 16)],
                        channels=128,
                        num_elems=ROWS,
                        d=4,
                        num_idxs=CH,
                    )
                    wb = gw.tile([128, CH, 4], BF16, tag="wb", name="wb")
                    wsrc = bass.AP(tensor=wflat_dram.tensor,
                                   offset=(wflat_dram.offset + k * 4 * HW
                                           + c0 * 4 * CH),
                                   ap=[[0, 128], [1, 4 * CH]])
                    wdst = bass.AP(tensor=wb.tensor, offset=wb.offset,
                                   ap=[[wb.ap[0][0], 128], [1, 4 * CH]])
                    nc.sync.dma_start(out=wdst, in_=wsrc)
                    TT_MUL(g, g, wb)
                    a2 = gw.tile([128, CH, 2], BF16, tag="a2", name="a2")
                    TT_ADD(a2, g[:, :, 0:2], g[:, :, 2:4])
                    col_k = colp.tile([128, CH], BF16, tag=f"col{k}",
                                      name=f"col{k}")
                    TT_ADD(col_k, a2[:, :, 0], a2[:, :, 1])
                    for m in range(2):
                        for h in range(2):
                            nc.tensor.matmul(
                                ps[m][h][:],
                                gemm_sb[:, k, m * 128:(m + 1) * 128],
                                col_k[:, h * 512:(h + 1) * 512],
                                start=(k == 0), stop=(k == K2 - 1),
                            )
                for m in range(2):
                    o_sb = outp.tile([128, CH], F32, tag=f"o{m}", name=f"o{m}")
                    for h in range(2):
                        TS(out=o_sb[:, h * 512:(h + 1) * 512], in0=ps[m][h],
                           scalar1=bias_sb[:, m:m + 1], scalar2=None,
                           op0=Alu.add)
                    dst = bass.AP(tensor=y_out,
                                  offset=m * 128 * HW + c0 * CH,
                                  ap=[[HW, 128], [1, CH]])
                    nc.sync.dma_start(out=dst, in_=o_sb)


def _host_constants():
    if "consts" in _CACHE:
        return _CACHE["consts"]
    t_idx = np.arange(NT)
    p_idx = np.arange(128)
    j = t_idx[None, :] * 128 + p_idx[:, None]          # [128, 32]
    iy = j // W
    ix = j % W
    ky = np.repeat(np.arange(K), K)
    kx = np.tile(np.arange(K), K)
    basey = np.zeros((128, KT), dtype=np.float32)
    basex = np.zeros((128, KT), dtype=np.float32)
    for k in range(K2):
        basey[:, k * NT:(k + 1) * NT] = iy - 1 + ky[k]
        basex[:, k * NT:(k + 1) * NT] = ix - 1 + kx[k]
    _CACHE["consts"] = (basey, basex)
    return _CACHE["consts"]


def kernel(x, offset_w, offset_b, mask_w, mask_b, weights, bias):
    x = np.asarray(x, dtype=np.float32)
    offset_w = np.asarray(offset_w, dtype=np.float32)
    mask_w = np.asarray(mask_w, dtype=np.float32)
    weights = np.asarray(weights, dtype=np.float32)
    bias = np.asarray(bias, dtype=np.float32)
    offset_b = np.asarray(offset_b, dtype=np.float32)
    mask_b = np.asarray(mask_b, dtype=np.float32)
    assert np.all(offset_b == 0) and np.all(mask_b == 0), "zero conv bias assumed"

    if "nc" not in _CACHE:
        _CACHE["nc"] = _build_nc()
    nc = _CACHE["nc"]
    basey, basex = _host_constants()

    # offset/mask conv stationary operand [c, tap, q]: q 0-8 dy, 9-17 dx, 18-26 m
    lhsT_om = np.zeros((C, K2, 32), dtype=np.float32)
    ow = offset_w.reshape(K2, 2, C, K, K)
    for tap in range(K2):
        dy, dx = tap // K, tap % K
        lhsT_om[:, tap, 0:K2] = ow[:, 0, :, dy, dx].T
        lhsT_om[:, tap, K2:2 * K2] = ow[:, 1, :, dy, dx].T
        lhsT_om[:, tap, 2 * K2:3 * K2] = mask_w[:, :, dy, dx].T
    lhsT_om = lhsT_om.astype(ml_dtypes.bfloat16)

    # GEMM stationary operand: lhsT_gemm[c, k, p] = weights[p, c*9 + k]
    wr = weights.reshape(P, C, K2)
    lhsT_gemm = np.ascontiguousarray(wr.transpose(1, 2, 0)).astype(ml_dtypes.bfloat16)

    bias_col = np.ascontiguousarray(bias.reshape(2, 128).T).astype(np.float32)

    in_maps = []
    for n in range(N):
        in_maps.append({
            "x": np.ascontiguousarray(x[n].reshape(C, HW)),
            "lhsT_om": lhsT_om,
            "lhsT_gemm": lhsT_gemm,
            "basey": basey,
            "basex": basex,
            "bias_col": bias_col,
        })

    res = run_bass_kernel_spmd(nc, in_maps, core_ids=list(range(N)),
                               trace=bool(_CACHE.get("trace")),
                               trace_cores=_CACHE.get("trace_cores"))
    _CACHE["last_res"] = res
    # device emits i-order columns (i = jp*32 + t <-> j = t*128 + jp)
    out = np.stack([res.results[n]["y"] for n in range(N)])
    out = out.reshape(N, P, 128, NT).transpose(0, 1, 3, 2).reshape(N, P, H, W)
    return np.ascontiguousarray(out.astype(np.float32))
